# revision 4
# baseline (speedup 1.0000x reference)
"""CSR Linear kernel for TRN2: out = x @ W^T + bias, W from COO nonzeros.

Strategy: data-parallel over tokens across 8 NeuronCores. Host densifies the
sparse weight into WT[in, out] (duplicate coords summed) and transposes x;
each core computes its 1024-token shard with a tiled f32r (TF32) matmul:
WT streamed from HBM once, x^T resident in SBUF, bias fused into the
PSUM->SBUF eviction.
"""

import os
import sys
import types

import numpy as np

TOKENS = 8192
IN_F = 4096
OUT_F = 4096
N_CORES = 8
P = 128

_CACHE = {}


def _ensure_ntff_hook():
    """Register the axon NTFF profile hook if the antenv stub lacks it.

    Only needed when tracing (BASS_TRACE=1); harmless otherwise. In
    environments with a real antenv.axon_hooks this is a no-op.
    """
    try:
        import antenv.axon_hooks  # noqa: F401

        return
    except ImportError:
        pass
    try:
        import antenv
        from trn_agent_boot.trn_boot import _ntff_profile_via_ctypes

        hooks = types.ModuleType("antenv.axon_hooks")
        hooks._hook = _ntff_profile_via_ctypes("/opt/axon/libaxon_pjrt.so")
        hooks.set_axon_ntff_profile_hook = lambda h: setattr(hooks, "_hook", h)
        hooks.get_axon_ntff_profile_hook = lambda: hooks._hook
        sys.modules["antenv.axon_hooks"] = hooks
        antenv.axon_hooks = hooks
    except Exception:
        pass


def _patch_upload():
    """Make trace artifact upload fall back to the local tmpdir when no
    artifact bucket is reachable (container environments)."""
    from concourse import bass_utils

    orig = bass_utils.upload_artifacts
    if getattr(orig, "_kernel_patched", False):
        return

    def _safe_upload(tmpdir):
        try:
            return orig(tmpdir)
        except Exception:
            return tmpdir

    _safe_upload._kernel_patched = True
    bass_utils.upload_artifacts = _safe_upload


def build_program(tok_per_core=TOKENS // N_CORES, in_f=IN_F, out_f=OUT_F):
    """Build + compile the per-core Bass program.

    out[tok_per_core, out_f] = xt.T @ wt + bias, with
      xt [in_f, tok_per_core] (f32r), wt [in_f, out_f] (f32r),
      biasr [128, out_f] (f32, pre-replicated across partitions).
    """
    key = (tok_per_core, in_f, out_f)
    if key in _CACHE:
        return _CACHE[key]

    import concourse.bacc as bacc
    import concourse.mybir as mybir
    import concourse.tile as tile

    N_TILE = 512  # out-feature block per psum bank
    KO = in_f // P  # k tiles
    M = tok_per_core // P  # token tiles
    NB = out_f // N_TILE  # out-feature blocks
    KO_CHUNK = 4  # k-tiles per WT DMA (1 MiB transfers)

    nc = bacc.Bacc("TRN2", target_bir_lowering=False, debug=False)

    xt = nc.dram_tensor("xt", [in_f, tok_per_core], mybir.dt.float32r, kind="ExternalInput")
    wt = nc.dram_tensor("wt", [in_f, out_f], mybir.dt.float32r, kind="ExternalInput")
    biasr = nc.dram_tensor("biasr", [P, out_f], mybir.dt.float32, kind="ExternalInput")
    out = nc.dram_tensor("out", [tok_per_core, out_f], mybir.dt.float32, kind="ExternalOutput")

    xt_ap = xt.ap().rearrange("(ko p) t -> p ko t", p=P)  # [P, KO, T]
    wt_ap = wt.ap().rearrange("(ko p) o -> p ko o", p=P)  # [P, KO, out_f]
    out_ap = out.ap().rearrange("(mo p) o -> p mo o", p=P)  # [P, M, out_f]

    with tile.TileContext(nc) as tc:
        WT_BUFS = 5
        with (
            tc.tile_pool(name="xt_pool", bufs=1) as xt_pool,
            tc.tile_pool(name="bias_pool", bufs=1) as bias_pool,
            tc.tile_pool(name="wt_pool", bufs=WT_BUFS) as wt_pool,
            tc.tile_pool(name="out_pool", bufs=4) as out_pool,
            tc.tile_pool(name="psum", bufs=8, space="PSUM") as psum_pool,
        ):
            xt_sb = xt_pool.tile([P, KO, tok_per_core], mybir.dt.float32r)

            def load_wt(n, kb):
                kbe = min(KO, kb + KO_CHUNK)
                ns = slice(n * N_TILE, (n + 1) * N_TILE)
                wt_t = wt_pool.tile(
                    [P, KO_CHUNK, N_TILE],
                    mybir.dt.float32r,
                    name=f"wt_{n}_{kb}",
                    tag="wt",
                )
                nc.sync.dma_start(wt_t[:, : kbe - kb, :], wt_ap[:, kb:kbe, ns])
                return wt_t

            # Prefetch the first n-block's WT chunks interleaved with the
            # leading x^T chunks so the PE isn't gated on either bulk load.
            ld = max(1, (1 << 20) // (P * tok_per_core * 4))  # k-tiles per ~1MiB DMA
            preloaded = {(0, 0): load_wt(0, 0)}
            nc.sync.dma_start(xt_sb[:, 0:ld, :], xt_ap[:, 0:ld, :])
            for kb in range(KO_CHUNK, min(WT_BUFS * KO_CHUNK, KO), KO_CHUNK):
                preloaded[(0, kb)] = load_wt(0, kb)

            # Resident x^T: [P, KO, T] f32r.
            for j in range(ld, KO, ld):
                je = min(KO, j + ld)
                nc.sync.dma_start(xt_sb[:, j:je, :], xt_ap[:, j:je, :])

            bias_sb = bias_pool.tile([P, out_f], mybir.dt.float32)
            nc.sync.dma_start(bias_sb[:], biasr.ap())

            for n in range(NB):
                ns = slice(n * N_TILE, (n + 1) * N_TILE)
                ps = [
                    psum_pool.tile(
                        [P, N_TILE], mybir.dt.float32, name=f"ps_{n}_{m}", tag="ps"
                    )
                    for m in range(M)
                ]
                for kb in range(0, KO, KO_CHUNK):
                    kbe = min(KO, kb + KO_CHUNK)
                    wt_t = preloaded.pop((n, kb), None)
                    if wt_t is None:
                        wt_t = load_wt(n, kb)
                    for kk in range(kbe - kb):
                        ko = kb + kk
                        for m in range(M):
                            nc.tensor.matmul(
                                ps[m][:],
                                lhsT=xt_sb[:, ko, m * P : (m + 1) * P],
                                rhs=wt_t[:, kk, :],
                                start=(ko == 0),
                                stop=(ko == KO - 1),
                            )
                for m in range(M):
                    ot = out_pool.tile(
                        [P, N_TILE], mybir.dt.float32, name=f"ot_{n}_{m}", tag="ot"
                    )
                    nc.vector.tensor_add(out=ot[:], in0=ps[m][:], in1=bias_sb[:, ns])
                    nc.sync.dma_start(out_ap[:, m, ns], ot[:])

    nc.compile()
    _CACHE[key] = nc
    return nc


def _densify_wt(values, row_ids, col_ids, in_f=IN_F, out_f=OUT_F):
    """WT[i, o] = sum of values[k] over k with col_ids[k]==i, row_ids[k]==o."""
    idx = col_ids.astype(np.int64) * out_f + row_ids.astype(np.int64)
    wt = np.bincount(idx, weights=values.astype(np.float64), minlength=in_f * out_f)
    return np.ascontiguousarray(wt.astype(np.float32).reshape(in_f, out_f))


def kernel(x, values, row_ids, col_ids, bias):
    from concourse import bass_utils

    if os.environ.get("BASS_TRACE"):
        _ensure_ntff_hook()
        _patch_upload()

    nc = build_program()

    wt = _densify_wt(values, row_ids, col_ids)
    bias_rep = np.ascontiguousarray(
        np.broadcast_to(bias.astype(np.float32), (P, OUT_F))
    )
    tpc = TOKENS // N_CORES
    in_maps = []
    for c in range(N_CORES):
        xt_c = np.ascontiguousarray(x[c * tpc : (c + 1) * tpc, :].T)
        in_maps.append({"xt": xt_c, "wt": wt, "biasr": bias_rep})

    res = bass_utils.run_bass_kernel_spmd(nc, in_maps, core_ids=list(range(N_CORES)))
    global last_results
    last_results = res
    return np.concatenate([res.results[c]["out"] for c in range(N_CORES)], axis=0)


last_results = None


# revision 5
# speedup vs baseline: 1.0261x; 1.0261x over previous
"""CSR Linear kernel for TRN2: out = x @ W^T + bias, W from COO nonzeros.

Strategy: data-parallel over tokens across 8 NeuronCores. Host densifies the
sparse weight into WT[in, out] (duplicate coords summed) and transposes x;
each core computes its 1024-token shard with a tiled f32r (TF32) matmul:
WT streamed from HBM once, x^T resident in SBUF, bias fused into the
PSUM->SBUF eviction.
"""

import os
import sys
import types

import numpy as np

TOKENS = 8192
IN_F = 4096
OUT_F = 4096
N_CORES = 8
P = 128

_CACHE = {}


def _ensure_ntff_hook():
    """Register the axon NTFF profile hook if the antenv stub lacks it.

    Only needed when tracing (BASS_TRACE=1); harmless otherwise. In
    environments with a real antenv.axon_hooks this is a no-op.
    """
    try:
        import antenv.axon_hooks  # noqa: F401

        return
    except ImportError:
        pass
    try:
        import antenv
        from trn_agent_boot.trn_boot import _ntff_profile_via_ctypes

        hooks = types.ModuleType("antenv.axon_hooks")
        hooks._hook = _ntff_profile_via_ctypes("/opt/axon/libaxon_pjrt.so")
        hooks.set_axon_ntff_profile_hook = lambda h: setattr(hooks, "_hook", h)
        hooks.get_axon_ntff_profile_hook = lambda: hooks._hook
        sys.modules["antenv.axon_hooks"] = hooks
        antenv.axon_hooks = hooks
    except Exception:
        pass


def _patch_upload():
    """Make trace artifact upload fall back to the local tmpdir when no
    artifact bucket is reachable (container environments)."""
    from concourse import bass_utils

    orig = bass_utils.upload_artifacts
    if getattr(orig, "_kernel_patched", False):
        return

    def _safe_upload(tmpdir):
        try:
            return orig(tmpdir)
        except Exception:
            return tmpdir

    _safe_upload._kernel_patched = True
    bass_utils.upload_artifacts = _safe_upload


def build_program(tok_per_core=TOKENS // N_CORES, in_f=IN_F, out_f=OUT_F):
    """Build + compile the per-core Bass program.

    out[tok_per_core, out_f] = xt.T @ wt + bias, with
      xt [in_f, tok_per_core] (f32r), wt [in_f, out_f] (f32r),
      biasr [128, out_f] (f32, pre-replicated across partitions).
    """
    key = (tok_per_core, in_f, out_f)
    if key in _CACHE:
        return _CACHE[key]

    import concourse.bacc as bacc
    import concourse.mybir as mybir
    import concourse.tile as tile

    N_TILE = 512  # out-feature block per psum bank
    KO = in_f // P  # k tiles
    M = tok_per_core // P  # token tiles
    NB = out_f // N_TILE  # out-feature blocks
    KO_CHUNK = 4  # k-tiles per WT DMA (1 MiB transfers)

    nc = bacc.Bacc("TRN2", target_bir_lowering=False, debug=False)

    xt = nc.dram_tensor("xt", [in_f, tok_per_core], mybir.dt.float32r, kind="ExternalInput")
    wt = nc.dram_tensor("wt", [in_f, out_f], mybir.dt.float32r, kind="ExternalInput")
    biasr = nc.dram_tensor("biasr", [P, out_f], mybir.dt.float32, kind="ExternalInput")
    out = nc.dram_tensor("out", [tok_per_core, out_f], mybir.dt.float32, kind="ExternalOutput")

    xt_ap = xt.ap().rearrange("(ko p) t -> p ko t", p=P)  # [P, KO, T]
    wt_ap = wt.ap().rearrange("(ko p) o -> p ko o", p=P)  # [P, KO, out_f]
    out_ap = out.ap().rearrange("(mo p) o -> p mo o", p=P)  # [P, M, out_f]

    with tile.TileContext(nc) as tc:
        WT_BUFS = 5
        with (
            tc.tile_pool(name="xt_pool", bufs=1) as xt_pool,
            tc.tile_pool(name="bias_pool", bufs=1) as bias_pool,
            tc.tile_pool(name="wt_pool", bufs=WT_BUFS) as wt_pool,
            tc.tile_pool(name="out_pool", bufs=4) as out_pool,
            tc.tile_pool(name="psum", bufs=8, space="PSUM") as psum_pool,
        ):
            xt_sb = xt_pool.tile([P, KO, tok_per_core], mybir.dt.float32r)

            def load_wt(n, kb):
                kbe = min(KO, kb + KO_CHUNK)
                ns = slice(n * N_TILE, (n + 1) * N_TILE)
                wt_t = wt_pool.tile(
                    [P, KO_CHUNK, N_TILE],
                    mybir.dt.float32r,
                    name=f"wt_{n}_{kb}",
                    tag="wt",
                )
                nc.sync.dma_start(wt_t[:, : kbe - kb, :], wt_ap[:, kb:kbe, ns])
                return wt_t

            # Prefetch the first n-block's WT chunks interleaved 1:1 with the
            # leading x^T chunks so the PE is gated on neither bulk load and
            # both streams stay proportionally fed during the first n-block.
            ld = min(KO, max(1, (1 << 20) // (P * tok_per_core * 4)))
            xt_chunks = list(range(0, KO, ld))
            wt_pre = list(range(0, min(WT_BUFS * KO_CHUNK, KO), KO_CHUNK))
            preloaded = {}
            xi = 0
            for kb in wt_pre:
                preloaded[(0, kb)] = load_wt(0, kb)
                if xi < len(xt_chunks):
                    j = xt_chunks[xi]
                    je = min(KO, j + ld)
                    nc.sync.dma_start(xt_sb[:, j:je, :], xt_ap[:, j:je, :])
                    xi += 1
            for j in xt_chunks[xi:]:
                je = min(KO, j + ld)
                nc.sync.dma_start(xt_sb[:, j:je, :], xt_ap[:, j:je, :])

            bias_sb = bias_pool.tile([P, out_f], mybir.dt.float32)
            nc.sync.dma_start(bias_sb[:], biasr.ap())

            for n in range(NB):
                ns = slice(n * N_TILE, (n + 1) * N_TILE)
                ps = [
                    psum_pool.tile(
                        [P, N_TILE], mybir.dt.float32, name=f"ps_{n}_{m}", tag="ps"
                    )
                    for m in range(M)
                ]
                for kb in range(0, KO, KO_CHUNK):
                    kbe = min(KO, kb + KO_CHUNK)
                    wt_t = preloaded.pop((n, kb), None)
                    if wt_t is None:
                        wt_t = load_wt(n, kb)
                    for kk in range(kbe - kb):
                        ko = kb + kk
                        for m in range(M):
                            nc.tensor.matmul(
                                ps[m][:],
                                lhsT=xt_sb[:, ko, m * P : (m + 1) * P],
                                rhs=wt_t[:, kk, :],
                                start=(ko == 0),
                                stop=(ko == KO - 1),
                            )
                for m in range(M):
                    ot = out_pool.tile(
                        [P, N_TILE], mybir.dt.float32, name=f"ot_{n}_{m}", tag="ot"
                    )
                    nc.vector.tensor_add(out=ot[:], in0=ps[m][:], in1=bias_sb[:, ns])
                    nc.sync.dma_start(out_ap[:, m, ns], ot[:])

    nc.compile()
    _CACHE[key] = nc
    return nc


def _densify_wt(values, row_ids, col_ids, in_f=IN_F, out_f=OUT_F):
    """WT[i, o] = sum of values[k] over k with col_ids[k]==i, row_ids[k]==o."""
    idx = col_ids.astype(np.int64) * out_f + row_ids.astype(np.int64)
    wt = np.bincount(idx, weights=values.astype(np.float64), minlength=in_f * out_f)
    return np.ascontiguousarray(wt.astype(np.float32).reshape(in_f, out_f))


def kernel(x, values, row_ids, col_ids, bias):
    from concourse import bass_utils

    if os.environ.get("BASS_TRACE"):
        _ensure_ntff_hook()
        _patch_upload()

    nc = build_program()

    wt = _densify_wt(values, row_ids, col_ids)
    bias_rep = np.ascontiguousarray(
        np.broadcast_to(bias.astype(np.float32), (P, OUT_F))
    )
    tpc = TOKENS // N_CORES
    in_maps = []
    for c in range(N_CORES):
        xt_c = np.ascontiguousarray(x[c * tpc : (c + 1) * tpc, :].T)
        in_maps.append({"xt": xt_c, "wt": wt, "biasr": bias_rep})

    res = bass_utils.run_bass_kernel_spmd(nc, in_maps, core_ids=list(range(N_CORES)))
    global last_results
    last_results = res
    return np.concatenate([res.results[c]["out"] for c in range(N_CORES)], axis=0)


last_results = None
